# revision 58
# baseline (speedup 1.0000x reference)
"""Bass/Tile TRN2 kernel: pairwise-MLP multi-head attention (B=2,T=256,C=128,H=4,HS=32).

Sharding: 8 cores = (batch b in {0,1}) x (query residue k in {0..3}); core
(b, k) owns the 64 queries i == k (mod 4), so every core sees the same mix of
causal extents (32 queries with j<128, 32 with j<256) and the SPMD program is
identical across cores.

Per-core dataflow:
  pre[c,(ip,j)] = fp8 DoubleRow matmuls: (W1p_lo | W1k) and (W1p_hi | W1q)
                  each contract 256 rows in one PE instruction (0.5 cyc/col).
                  pos_dist arrives channel-major (pre-transposed + fp8 on CPU),
                  x1-key and x1-query broadcasts are prebuilt fp8 columns so the
                  kt-jump access pattern pairs them with the pdt halves.
  g = gelu(pre + b1[h])                        (ScalarE, psum -> sbuf bf16)
  score_t[j,i] = g_chunk.T @ w2[h]             (PE, g stationary, bf16)
  P_t = exp(scale*score_t + b2*scale) * mask   (ScalarE + DVE)
  out[i,:] = P_t.T @ [v | 1]; out /= Z         (PE; Z rides as v's 33rd column)

Causal skip: queries i<128 only compute/load the j<128 half -> 25% less PE,
Scalar and DMA work, identically on every core.
"""

import sys
from contextlib import ExitStack

import numpy as np

for _p in ("/opt/trn_rl_repo", "/root/.axon_site/_ro/trn_rl_repo"):
    if _p not in sys.path:
        sys.path.append(_p)

import ml_dtypes

import concourse.bass as bass
import concourse.mybir as mybir
import concourse.tile as tile
from concourse.bass_utils import run_bass_kernel_spmd

B, T, C = 2, 256, 128
H, HS = 4, 32
IBLK = 64            # queries per core
NCORES = 8
SCALE = float(C) ** -0.5

F32 = mybir.dt.float32
BF16 = mybir.dt.bfloat16
F8 = mybir.dt.float8e4
DR = mybir.MatmulPerfMode.DoubleRow

GELU = mybir.ActivationFunctionType.Gelu
EXP = mybir.ActivationFunctionType.Exp

# big-tile column layout (fp8, per partition).  Query extents are rounded to
# {64, 128, 256}: local q 0-15 (i<64) -> 64, q 16-31 -> 128, q 32-63 -> 256.
# x1 key rows sit next to the pdt regions they pair with so every kt-jump
# stride fits the 16-bit ISA step field, and everything the first groups need
# is one contiguous wave-1 chunk [0:P0L].
P0A = 0                  # pdt kt0, xs pairs (8 x 2q x 64j)
P0B = 1024               # pdt kt0, s pairs (8 x 2q x 128j)
XT64 = 3072              # x1 key row j<64, duplicated, tiled x4  (512 cols)
XT128 = 3584             # x1 key row j<128, duplicated, tiled x2 (512 cols)
P1A = 4096               # pdt kt1, xs pairs
P1B = 5120               # pdt kt1, s pairs
P0L = 7168               # pdt kt0, long pairs (16 x 2q x 256j)
XT256 = 15360            # x1 key row full, duplicated            (512 cols)
P1L = 15872              # pdt kt1, long pairs
XQA = 24064              # x1 query-broadcast, xs pair layout  (on-chip)
XQS = 25088              # x1 query-broadcast, s pairs
XQL = 27136              # x1 query-broadcast, long pairs
NBIG = 35328
DB_XS = XQA - P1A        # 19968: kt-jump for the (W1p_hi | W1q) matmul
DB_L = XQL - P1L         # 11264

NWQ = 2048 + IBLK + 8 + 16

_build_cache = {}


def _legalize_single_wait(bir_json):
    """Split multi-wait instructions into single-wait NoOps + instruction.

    This walrus build's codegen (setupSyncWait) accepts at most one sem wait
    per ISA struct, but Tile's sem-assignment attaches wait *lists*.  Waits
    are ANDed and executed in order by the issuing sequencer, so hoisting all
    but one onto same-engine NoOps immediately before is semantically
    identical.
    """
    import json as _json

    m = _json.loads(bir_json)
    for fn in m.get("functions", []):
        for blk in fn.get("blocks", []):
            new = []
            for ins in blk.get("instructions", []):
                si = ins.get("sync_info")
                waits = (si or {}).get("on_wait") or []
                if len(waits) > 1:
                    for k, w in enumerate(waits[:-1]):
                        nop = {
                            "debug": ins.get("debug", 0),
                            "engine": ins["engine"],
                            "ins": [],
                            "name": f"{ins['name']}-ws{k}",
                            "opcode": "NoOp",
                            "outs": [],
                            "sync_info": {"on_wait": [w], "on_update": []},
                        }
                        new.append(nop)
                    si = dict(si)
                    si["on_wait"] = [waits[-1]]
                    ins = dict(ins)
                    ins["sync_info"] = si
                new.append(ins)
            blk["instructions"] = new
    return _json.dumps(m).encode()


def _install_wait_legalizer():
    from concourse import bass2jax as _b2j
    from concourse import bass_utils as _bu

    if getattr(_b2j, "_single_wait_patched", False):
        return
    _orig = _bu.compile_bir_kernel

    def _patched(bir_json, tmpdir, neff_name="file.neff"):
        return _orig(_legalize_single_wait(bir_json), tmpdir, neff_name)

    _b2j.compile_bir_kernel = _patched
    _b2j._single_wait_patched = True


def _bcast_ap(ap2d, count, pos):
    """Insert a [0, count] broadcast dim at free position `pos`."""
    dims = [list(d) for d in ap2d.ap]
    dims.insert(pos, [0, count])
    return bass.AP(tensor=ap2d.tensor, offset=ap2d.offset, ap=dims)


def _ktjump(tile_ap, off, delta, ncols):
    """rhs AP [128][kt: stride delta, 2][1, ncols] rooted at column `off`."""
    sl = tile_ap[:, off : off + 1]
    return bass.AP(
        tensor=sl.tensor,
        offset=sl.offset,
        ap=[list(sl.ap[0]), [delta, 2], [1, ncols]],
    )


def _build(b2_scaled):
    nc = bass.Bass()

    # DMA carries only pdt + x1 rows (XQ* is pure replication, built on-chip)
    big = nc.dram_tensor("big", (128, XQA), F8, kind="ExternalInput")
    # wabq: wab [c, a/b, kt, h, m] flat | xq | w2 (bf16 bits) | b1 (f32 bits)
    # -> one wave-1 chunk carries every weight the first groups need.
    wabq = nc.dram_tensor("wabq", (128, NWQ), F8, kind="ExternalInput")
    cpack = nc.dram_tensor("cpack", (128, 516), BF16, kind="ExternalInput")
    out = nc.dram_tensor("out", (IBLK, H * HS), F32, kind="ExternalOutput")

    with tile.TileContext(nc) as tc, ExitStack() as ctx:
        const = ctx.enter_context(tc.tile_pool(name="const", bufs=1))
        gpool = ctx.enter_context(tc.tile_pool(name="gpool", bufs=4))
        psg = ctx.enter_context(tc.tile_pool(name="psg", bufs=2, space="PSUM"))
        pssc = ctx.enter_context(tc.tile_pool(name="pssc", bufs=1, space="PSUM"))
        psepi = ctx.enter_context(tc.tile_pool(name="psepi", bufs=1, space="PSUM"))

        # ---------- constants (DMAs spread over the 3 DMA-capable queues so
        # triggers (~0.6us each) and transfers overlap; each queue issues its
        # first-group-critical chunks first) ----------
        wq_sb = const.tile([128, NWQ], F8)
        cp_sb = const.tile([128, 516], BF16)
        big_sb = const.tile([128, NBIG], F8)

        msl = cp_sb[:, 388:389]
        mask_bc = bass.AP(tensor=msl.tensor, offset=msl.offset,
                          ap=[list(msl.ap[0]), [64, 2], [0, H], [1, 64]])
        def w2_ap(h):
            o = 2048 + IBLK + 2 * h
            return wq_sb[:, o : o + 2].bitcast(BF16)

        def b1_ap(h):
            o = 2048 + IBLK + 8 + 4 * h
            return wq_sb[:, o : o + 4].bitcast(F32)

        def wsl(ab, h):
            """lhsT view [c, kt, 128] for the a/b weight pack of head h."""
            sl = wq_sb[:, ab * 1024 + h * 128 : ab * 1024 + h * 128 + 1]
            return bass.AP(tensor=sl.tensor, offset=sl.offset,
                           ap=[list(sl.ap[0]), [512, 2], [1, 128]])

        # Wave 1 (parallel queues): everything the first long groups need
        # (long groups run first so the PE ramps to full clock before the
        # instruction-denser short groups).  Later waves: the rest.
        nc.gpsimd.dma_start(out=wq_sb, in_=wabq[:])
        nc.scalar.dma_start(out=big_sb[:, P0L : P0L + 4096],
                            in_=big[:, P0L : P0L + 4096])
        nc.sync.dma_start(out=big_sb[:, XT256 : XT256 + 4608],
                          in_=big[:, XT256 : XT256 + 4608])
        nc.gpsimd.dma_start(out=big_sb[:, P0L + 4096 : XT256],
                            in_=big[:, P0L + 4096 : XT256])
        nc.sync.dma_start(out=big_sb[:, XT256 + 4608 : XQA],
                          in_=big[:, XT256 + 4608 : XQA])
        nc.scalar.dma_start(out=big_sb[:, 0:P0L], in_=big[:, 0:P0L])
        nc.sync.dma_start(out=cp_sb, in_=cpack[:])

        # x1 query-broadcast columns: replicate on-chip (DVE), in the order
        # the pair loop consumes them (long pairs first).
        for p in list(range(16, 32)) + list(range(16)):
            if p < 8:
                off, ext = XQA + p * 128, 64
            elif p < 16:
                off, ext = XQS + (p - 8) * 256, 128
            else:
                off, ext = XQL + (p - 16) * 512, 256
            src = wq_sb[:, 2048 + 2 * p : 2048 + 2 * p + 1]
            src_bc = bass.AP(tensor=src.tensor, offset=src.offset,
                             ap=[list(src.ap[0]), [1, 2], [0, ext]])
            dst = big_sb[:, off : off + 1]
            dst_ap = bass.AP(tensor=dst.tensor, offset=dst.offset,
                             ap=[list(dst.ap[0]), [ext, 2], [1, ext]])
            nc.vector.tensor_copy(dst_ap, src_bc)

        pt_sb = const.tile([128, 2, H, IBLK], BF16)
        v_sb = const.tile([128, 2, H, HS + 1], BF16)
        recip = const.tile([IBLK, H, 1], F32)
        final_sb = const.tile([IBLK, H * HS], F32)

        # score accumulator [j%128, jb, h, i] - 1 PSUM bank, memset so the
        # never-written j-tiles of short queries exp() to a finite value.
        score_ps = pssc.tile([128, 2, H, IBLK], F32)
        nc.vector.memset(score_ps, 0.0)
        nc.vector.memset(v_sb[:, :, :, HS : HS + 1], 1.0)

        # ---------- v = x @ Wv (+ ones column for Z) ----------
        for jc in range(2):
            v_ps = psepi.tile([128, H, HS], F32, tag="vps", name=f"v{jc}")
            nc.tensor.matmul(v_ps, lhsT=cp_sb[:, 132 + jc * 128 : 260 + jc * 128],
                             rhs=cp_sb[:, 4:132], start=True, stop=True)
            nc.vector.tensor_copy(v_sb[:, jc, :, 0:HS], v_ps)

        # ---------- main loop ----------
        # groups (each fills one [128, 8, 128] psum tile = 1024 columns):
        #   "x": 8 pairs ext 64 (q 0-15), "s": 4 pairs ext 128 (q 16-31),
        #   "l": 2 pairs ext 256 (q 32-63)
        groups = [("l", g) for g in range(8)] + [("x", 0), ("s", 0), ("s", 1)]
        pending = []

        def emit_scores(g_t, kind, gi, h):
            if kind == "x":
                for qi in range(16):
                    nc.tensor.matmul(
                        score_ps[0:64, 0, h, qi : qi + 1],
                        lhsT=g_t[:, qi // 2, (qi % 2) * 64 : (qi % 2) * 64 + 64],
                        rhs=w2_ap(h), start=True, stop=True)
            elif kind == "s":
                for qi in range(8):
                    q = 16 + 8 * gi + qi
                    nc.tensor.matmul(
                        score_ps[:, 0, h, q : q + 1],
                        lhsT=g_t[:, qi, :], rhs=w2_ap(h), start=True, stop=True)
            else:
                for qi in range(4):
                    q = 32 + 4 * gi + qi
                    for jb in range(2):
                        nc.tensor.matmul(
                            score_ps[:, jb, h, q : q + 1],
                            lhsT=g_t[:, 2 * qi + jb, :],
                            rhs=w2_ap(h), start=True, stop=True)

        for kind, gi in groups:
            # each block = one N=512 DoubleRow matmul pair covering 4 psum rows
            if kind == "x":
                plist = [(b, big_sb, 512 * b, XT64 - 512 * b, DB_XS, P1A)
                         for b in range(2)]
            elif kind == "s":
                plist = [(b, big_sb, P0B + 1024 * gi + 512 * b, 0, DB_XS, P1A)
                         for b in range(2)]
                plist = [(b, t, o0, XT128 - o0, db, d1)
                         for b, t, o0, _, db, d1 in plist]
            else:
                plist = []
                for pp in range(2):
                    o0 = P0L + (2 * gi + pp) * 512
                    plist.append((pp, big_sb, o0, XT256 - o0, DB_L, P1L))
            for h in range(H):
                ps = psg.tile([128, 8, 128], F32, tag="pre", name=f"ps{kind}{gi}_{h}")
                for b, tl, o0, da, db, p1 in plist:
                    out_sl = ps[:, 4 * b : 4 * b + 4, :]
                    d1 = p1 - (P0A if kind != "l" else P0L)
                    nc.tensor.matmul(out_sl, lhsT=wsl(0, h),
                                     rhs=_ktjump(tl, o0, da, 512),
                                     start=True, stop=False, perf_mode=DR)
                    nc.tensor.matmul(out_sl, lhsT=wsl(1, h),
                                     rhs=_ktjump(tl, o0 + d1, db, 512),
                                     start=False, stop=True, perf_mode=DR)
                g_t = gpool.tile([128, 8, 128], BF16, tag="g", name=f"g{kind}{gi}_{h}")
                nc.scalar.activation(out=g_t, in_=ps, func=GELU,
                                     bias=b1_ap(h), scale=1.0)
                pending.append((g_t, kind, gi, h))
                if len(pending) > 1:
                    emit_scores(*pending.pop(0))
        while pending:
            emit_scores(*pending.pop(0))

        # ---------- epilogue ----------
        if len(set(b2_scaled)) == 1:
            nc.scalar.activation(out=pt_sb, in_=score_ps, func=EXP,
                                 bias=float(b2_scaled[0]), scale=SCALE)
        else:
            for h in range(H):
                nc.scalar.activation(out=pt_sb[:, :, h, :],
                                     in_=score_ps[:, :, h, :],
                                     func=EXP, bias=float(b2_scaled[h]),
                                     scale=SCALE)
        nc.vector.tensor_mul(pt_sb, pt_sb, mask_bc)

        av = psepi.tile([IBLK, H, HS + 1], F32, tag="av", name="av")
        for h in range(H):
            for jc in range(2):
                nc.tensor.matmul(av[:, h, :], lhsT=pt_sb[:, jc, h, :],
                                 rhs=v_sb[:, jc, h, :],
                                 start=(jc == 0), stop=(jc == 1))
        nc.vector.reciprocal(out=recip, in_=av[:, :, HS : HS + 1])
        rb = recip[:, :, 0:1]
        recip_bc = bass.AP(tensor=rb.tensor, offset=rb.offset,
                           ap=[list(rb.ap[0]), [1, H], [0, HS]])
        fview = bass.AP(tensor=final_sb.tensor, offset=final_sb.offset,
                        ap=[list(final_sb.ap[0]), [HS, H], [1, HS]])
        nc.vector.tensor_mul(fview, av[:, :, 0:HS], recip_bc)
        nc.sync.dma_start(out=out[:], in_=final_sb)

    return nc


def _prep_core(x1t_b, pd_b, k):
    """Build the per-core big-tile columns (fp8) and mask for residue k."""
    f8 = ml_dtypes.float8_e4m3fn
    qsel = 4 * np.arange(IBLK) + k
    arr = pd_b[qsel].transpose(2, 0, 1)            # (256 c2, 64 q, 256 j)
    kt0, kt1 = arr[0:128], arr[128:256]
    xs0 = kt0[:, 0:16, 0:64].reshape(128, 1024)
    xs1 = kt1[:, 0:16, 0:64].reshape(128, 1024)
    s0 = kt0[:, 16:32, 0:128].reshape(128, 2048)
    s1 = kt1[:, 16:32, 0:128].reshape(128, 2048)
    l0 = kt0[:, 32:64, :].reshape(128, 8192)
    l1 = kt1[:, 32:64, :].reshape(128, 8192)
    x1qs = np.ascontiguousarray(x1t_b[:, qsel]).astype(f8)   # (128, 64)
    xt64 = np.tile(x1t_b[:, 0:64], (1, 8))
    xt128 = np.tile(x1t_b[:, 0:128], (1, 4))
    xt256 = np.tile(x1t_b, (1, 2))
    bigc = np.concatenate(
        [xs0, s0, xt64, xt128, xs1, s1, l0, xt256, l1], axis=1).astype(f8)
    jidx = np.arange(128)[:, None, None] + np.array([0, 128])[None, :, None]
    mask = (jidx <= (4 * np.arange(IBLK) + k)[None, None, :]).astype(
        ml_dtypes.bfloat16)
    return bigc, x1qs, mask


def kernel(**inputs):
    x = np.asarray(inputs["x"], np.float32)
    st = np.asarray(inputs["st_pos_emb"], np.float32)
    pd = np.asarray(inputs["pos_dist_emb"], np.float32)
    W1 = np.asarray(inputs["W1"], np.float32)
    b1 = np.asarray(inputs["b1"], np.float32)
    W2 = np.asarray(inputs["W2"], np.float32)
    b2 = np.asarray(inputs["b2"], np.float32)
    Wv = np.asarray(inputs["Wv"], np.float32)
    bv = np.asarray(inputs["bv"], np.float32)

    bf = ml_dtypes.bfloat16
    f8 = ml_dtypes.float8_e4m3fn
    x1 = x + st[None]                                    # (B, T, C)
    x1t_b = np.ascontiguousarray(x1.transpose(0, 2, 1))  # (B, C, T)

    W1k = W1[:, :C, :]                                   # (H, C, C)
    W1q = W1[:, C : 2 * C, :]
    W1p = W1[:, 2 * C :, :]                              # (H, 2C, C)
    wa_a = np.stack([W1p[:, 0:128, :], W1k], axis=0)     # (kt, H, c, m)
    wb_a = np.stack([W1p[:, 128:256, :], W1q], axis=0)
    wab_a = np.ascontiguousarray(
        np.stack([wa_a, wb_a], axis=0).transpose(3, 0, 1, 2, 4)
    ).astype(f8)                                         # (c, a/b, kt, h, m)
    w2_a = np.ascontiguousarray(W2.T).astype(bf)         # (C, H)
    b1_a = np.ascontiguousarray(b1.T)                    # (C, H)
    wv_a = Wv.transpose(1, 0, 2).reshape(C, H * HS).astype(bf)

    key = tuple(float(v) * SCALE for v in b2)
    if key not in _build_cache:
        _build_cache[key] = _build(key)
    nc = _build_cache[key]

    in_maps = []
    for core in range(NCORES):
        b, k = divmod(core, 4)
        bigc, x1qs, mask = _prep_core(x1t_b[b], pd[b], k)
        cpack = np.concatenate(
            [w2_a, wv_a, x[b].T.astype(bf), mask.reshape(128, 128)], axis=1)
        wabq = np.concatenate(
            [wab_a.reshape(128, 2048), x1qs, w2_a.view(f8), b1_a.view(f8)],
            axis=1)
        in_maps.append({
            "big": bigc, "wabq": np.ascontiguousarray(wabq),
            "cpack": np.ascontiguousarray(cpack),
        })

    _install_wait_legalizer()
    res = run_bass_kernel_spmd(nc, in_maps, core_ids=list(range(NCORES)))
    outp = np.zeros((B, T, H * HS), np.float32)
    for core in range(NCORES):
        b, k = divmod(core, 4)
        outp[b, 4 * np.arange(IBLK) + k] = res.results[core]["out"]
    outp += bv.reshape(-1)[None, None, :]
    return outp


# revision 59
# speedup vs baseline: 1.0293x; 1.0293x over previous
"""Bass/Tile TRN2 kernel: pairwise-MLP multi-head attention (B=2,T=256,C=128,H=4,HS=32).

Sharding: 8 cores = (batch b in {0,1}) x (query residue k in {0..3}); core
(b, k) owns the 64 queries i == k (mod 4), so every core sees the same mix of
causal extents (32 queries with j<128, 32 with j<256) and the SPMD program is
identical across cores.

Per-core dataflow:
  pre[c,(ip,j)] = fp8 DoubleRow matmuls: (W1p_lo | W1k) and (W1p_hi | W1q)
                  each contract 256 rows in one PE instruction (0.5 cyc/col).
                  pos_dist arrives channel-major (pre-transposed + fp8 on CPU),
                  x1-key and x1-query broadcasts are prebuilt fp8 columns so the
                  kt-jump access pattern pairs them with the pdt halves.
  g = gelu(pre + b1[h])                        (ScalarE, psum -> sbuf bf16)
  score_t[j,i] = g_chunk.T @ w2[h]             (PE, g stationary, bf16)
  P_t = exp(scale*score_t + b2*scale) * mask   (ScalarE + DVE)
  out[i,:] = P_t.T @ [v | 1]; out /= Z         (PE; Z rides as v's 33rd column)

Causal skip: queries i<128 only compute/load the j<128 half -> 25% less PE,
Scalar and DMA work, identically on every core.
"""

import sys
from contextlib import ExitStack

import numpy as np

for _p in ("/opt/trn_rl_repo", "/root/.axon_site/_ro/trn_rl_repo"):
    if _p not in sys.path:
        sys.path.append(_p)

import ml_dtypes

import concourse.bass as bass
import concourse.mybir as mybir
import concourse.tile as tile
from concourse.bass_utils import run_bass_kernel_spmd

B, T, C = 2, 256, 128
H, HS = 4, 32
IBLK = 64            # queries per core
NCORES = 8
SCALE = float(C) ** -0.5

F32 = mybir.dt.float32
BF16 = mybir.dt.bfloat16
F8 = mybir.dt.float8e4
DR = mybir.MatmulPerfMode.DoubleRow

GELU = mybir.ActivationFunctionType.Gelu
EXP = mybir.ActivationFunctionType.Exp

# big-tile column layout (fp8, per partition).  Query extents are rounded to
# {64, 128, 256}: local q 0-15 (i<64) -> 64, q 16-31 -> 128, q 32-63 -> 256.
# x1 key rows sit next to the pdt regions they pair with so every kt-jump
# stride fits the 16-bit ISA step field, and everything the first groups need
# is one contiguous wave-1 chunk [0:P0L].
P0A = 0                  # pdt kt0, xs pairs (8 x 2q x 64j)
P0B = 1024               # pdt kt0, s pairs (8 x 2q x 128j)
XT64 = 3072              # x1 key row j<64, duplicated, tiled x4  (512 cols)
XT128 = 3584             # x1 key row j<128, duplicated, tiled x2 (512 cols)
P1A = 4096               # pdt kt1, xs pairs
P1B = 5120               # pdt kt1, s pairs
P0L = 7168               # pdt kt0, long pairs (16 x 2q x 256j)
XT256 = 15360            # x1 key row full, duplicated            (512 cols)
P1L = 15872              # pdt kt1, long pairs
XQA = 24064              # x1 query-broadcast, xs pair layout  (on-chip)
XQS = 25088              # x1 query-broadcast, s pairs
XQL = 27136              # x1 query-broadcast, long pairs
NBIG = 35328
DB_XS = XQA - P1A        # 19968: kt-jump for the (W1p_hi | W1q) matmul
DB_L = XQL - P1L         # 11264

NWQ = 2048 + IBLK + 8 + 16

_build_cache = {}


def _legalize_single_wait(bir_json):
    """Split multi-wait instructions into single-wait NoOps + instruction.

    This walrus build's codegen (setupSyncWait) accepts at most one sem wait
    per ISA struct, but Tile's sem-assignment attaches wait *lists*.  Waits
    are ANDed and executed in order by the issuing sequencer, so hoisting all
    but one onto same-engine NoOps immediately before is semantically
    identical.
    """
    import json as _json

    m = _json.loads(bir_json)
    for fn in m.get("functions", []):
        for blk in fn.get("blocks", []):
            new = []
            for ins in blk.get("instructions", []):
                si = ins.get("sync_info")
                waits = (si or {}).get("on_wait") or []
                if len(waits) > 1:
                    for k, w in enumerate(waits[:-1]):
                        nop = {
                            "debug": ins.get("debug", 0),
                            "engine": ins["engine"],
                            "ins": [],
                            "name": f"{ins['name']}-ws{k}",
                            "opcode": "NoOp",
                            "outs": [],
                            "sync_info": {"on_wait": [w], "on_update": []},
                        }
                        new.append(nop)
                    si = dict(si)
                    si["on_wait"] = [waits[-1]]
                    ins = dict(ins)
                    ins["sync_info"] = si
                new.append(ins)
            blk["instructions"] = new
    return _json.dumps(m).encode()


def _install_wait_legalizer():
    from concourse import bass2jax as _b2j
    from concourse import bass_utils as _bu

    if getattr(_b2j, "_single_wait_patched", False):
        return
    _orig = _bu.compile_bir_kernel

    def _patched(bir_json, tmpdir, neff_name="file.neff"):
        return _orig(_legalize_single_wait(bir_json), tmpdir, neff_name)

    _b2j.compile_bir_kernel = _patched
    _b2j._single_wait_patched = True


def _bcast_ap(ap2d, count, pos):
    """Insert a [0, count] broadcast dim at free position `pos`."""
    dims = [list(d) for d in ap2d.ap]
    dims.insert(pos, [0, count])
    return bass.AP(tensor=ap2d.tensor, offset=ap2d.offset, ap=dims)


def _ktjump(tile_ap, off, delta, ncols):
    """rhs AP [128][kt: stride delta, 2][1, ncols] rooted at column `off`."""
    sl = tile_ap[:, off : off + 1]
    return bass.AP(
        tensor=sl.tensor,
        offset=sl.offset,
        ap=[list(sl.ap[0]), [delta, 2], [1, ncols]],
    )


def _build(b2_scaled):
    nc = bass.Bass()

    # DMA carries only pdt + x1 rows (XQ* is pure replication, built on-chip)
    big = nc.dram_tensor("big", (128, XQA), F8, kind="ExternalInput")
    # wabq: wab [c, a/b, kt, h, m] flat | xq | w2 (bf16 bits) | b1 (f32 bits)
    # -> one wave-1 chunk carries every weight the first groups need.
    wabq = nc.dram_tensor("wabq", (128, NWQ), F8, kind="ExternalInput")
    cpack = nc.dram_tensor("cpack", (128, 516), BF16, kind="ExternalInput")
    out = nc.dram_tensor("out", (IBLK, H * HS), F32, kind="ExternalOutput")

    with tile.TileContext(nc) as tc, ExitStack() as ctx:
        const = ctx.enter_context(tc.tile_pool(name="const", bufs=1))
        gpool = ctx.enter_context(tc.tile_pool(name="gpool", bufs=4))
        psg = ctx.enter_context(tc.tile_pool(name="psg", bufs=2, space="PSUM"))
        pssc = ctx.enter_context(tc.tile_pool(name="pssc", bufs=1, space="PSUM"))
        psepi = ctx.enter_context(tc.tile_pool(name="psepi", bufs=1, space="PSUM"))

        # ---------- constants (DMAs spread over the 3 DMA-capable queues so
        # triggers (~0.6us each) and transfers overlap; each queue issues its
        # first-group-critical chunks first) ----------
        wq_sb = const.tile([128, NWQ], F8)
        cp_sb = const.tile([128, 516], BF16)
        big_sb = const.tile([128, NBIG], F8)

        msl = cp_sb[:, 388:389]
        mask_bc = bass.AP(tensor=msl.tensor, offset=msl.offset,
                          ap=[list(msl.ap[0]), [64, 2], [0, H], [1, 64]])
        def w2_ap(h):
            o = 2048 + IBLK + 2 * h
            return wq_sb[:, o : o + 2].bitcast(BF16)

        def b1_ap(h):
            o = 2048 + IBLK + 8 + 4 * h
            return wq_sb[:, o : o + 4].bitcast(F32)

        def wsl(ab, h):
            """lhsT view [c, kt, 128] for the a/b weight pack of head h."""
            sl = wq_sb[:, ab * 1024 + h * 128 : ab * 1024 + h * 128 + 1]
            return bass.AP(tensor=sl.tensor, offset=sl.offset,
                           ap=[list(sl.ap[0]), [512, 2], [1, 128]])

        # Wave 1 (parallel queues): everything the first long groups need
        # (long groups run first so the PE ramps to full clock before the
        # instruction-denser short groups).  Later waves: the rest.
        nc.gpsimd.dma_start(out=wq_sb, in_=wabq[:])
        nc.scalar.dma_start(out=big_sb[:, P0L : P0L + 4096],
                            in_=big[:, P0L : P0L + 4096])
        nc.sync.dma_start(out=big_sb[:, XT256 : XT256 + 4608],
                          in_=big[:, XT256 : XT256 + 4608])
        nc.gpsimd.dma_start(out=big_sb[:, P0L + 4096 : XT256],
                            in_=big[:, P0L + 4096 : XT256])
        nc.sync.dma_start(out=big_sb[:, XT256 + 4608 : XQA],
                          in_=big[:, XT256 + 4608 : XQA])
        nc.scalar.dma_start(out=big_sb[:, 0:P0L], in_=big[:, 0:P0L])
        nc.sync.dma_start(out=cp_sb, in_=cpack[:])

        # x1 query-broadcast columns: replicate on-chip (DVE), in the order
        # the pair loop consumes them (long pairs first).
        for p in list(range(16, 32)) + list(range(16)):
            if p < 8:
                off, ext = XQA + p * 128, 64
            elif p < 16:
                off, ext = XQS + (p - 8) * 256, 128
            else:
                off, ext = XQL + (p - 16) * 512, 256
            src = wq_sb[:, 2048 + 2 * p : 2048 + 2 * p + 1]
            src_bc = bass.AP(tensor=src.tensor, offset=src.offset,
                             ap=[list(src.ap[0]), [1, 2], [0, ext]])
            dst = big_sb[:, off : off + 1]
            dst_ap = bass.AP(tensor=dst.tensor, offset=dst.offset,
                             ap=[list(dst.ap[0]), [ext, 2], [1, ext]])
            nc.vector.tensor_copy(dst_ap, src_bc)

        pt_sb = const.tile([128, 2, H, IBLK], BF16)
        v_sb = const.tile([128, 2, H, HS + 1], BF16)
        recip = const.tile([IBLK, H, 1], F32)
        final_sb = const.tile([IBLK, H * HS], F32)

        # score accumulator [j%128, jb, h, i] - 1 PSUM bank, memset so the
        # never-written j-tiles of short queries exp() to a finite value.
        score_ps = pssc.tile([128, 2, H, IBLK], F32)
        nc.vector.memset(score_ps, 0.0)
        nc.vector.memset(v_sb[:, :, :, HS : HS + 1], 1.0)

        # ---------- v = x @ Wv (+ ones column for Z) ----------
        for jc in range(2):
            v_ps = psepi.tile([128, H, HS], F32, tag="vps", name=f"v{jc}")
            nc.tensor.matmul(v_ps, lhsT=cp_sb[:, 132 + jc * 128 : 260 + jc * 128],
                             rhs=cp_sb[:, 4:132], start=True, stop=True)
            nc.vector.tensor_copy(v_sb[:, jc, :, 0:HS], v_ps)

        # ---------- main loop ----------
        # groups (each fills one [128, 8, 128] psum tile = 1024 columns):
        #   "x": 8 pairs ext 64 (q 0-15), "s": 4 pairs ext 128 (q 16-31),
        #   "l": 2 pairs ext 256 (q 32-63)
        groups = [("l", g) for g in range(8)] + [("x", 0), ("s", 0), ("s", 1)]
        pending = []

        def emit_scores(g_t, kind, gi, h):
            if kind == "x":
                for qi in range(16):
                    nc.tensor.matmul(
                        score_ps[0:64, 0, h, qi : qi + 1],
                        lhsT=g_t[:, qi // 2, (qi % 2) * 64 : (qi % 2) * 64 + 64],
                        rhs=w2_ap(h), start=True, stop=True)
            elif kind == "s":
                for qi in range(8):
                    q = 16 + 8 * gi + qi
                    nc.tensor.matmul(
                        score_ps[:, 0, h, q : q + 1],
                        lhsT=g_t[:, qi, :], rhs=w2_ap(h), start=True, stop=True)
            else:
                for qi in range(4):
                    q = 32 + 4 * gi + qi
                    for jb in range(2):
                        nc.tensor.matmul(
                            score_ps[:, jb, h, q : q + 1],
                            lhsT=g_t[:, 2 * qi + jb, :],
                            rhs=w2_ap(h), start=True, stop=True)

        for kind, gi in groups:
            # each block = one N=512 DoubleRow matmul pair covering 4 psum rows
            if kind == "x":
                plist = [(b, big_sb, 512 * b, XT64 - 512 * b, DB_XS, P1A)
                         for b in range(2)]
            elif kind == "s":
                plist = [(b, big_sb, P0B + 1024 * gi + 512 * b, 0, DB_XS, P1A)
                         for b in range(2)]
                plist = [(b, t, o0, XT128 - o0, db, d1)
                         for b, t, o0, _, db, d1 in plist]
            else:
                plist = []
                for pp in range(2):
                    o0 = P0L + (2 * gi + pp) * 512
                    plist.append((pp, big_sb, o0, XT256 - o0, DB_L, P1L))
            for h in range(H):
                ps = psg.tile([128, 8, 128], F32, tag="pre", name=f"ps{kind}{gi}_{h}")
                for b, tl, o0, da, db, p1 in plist:
                    out_sl = ps[:, 4 * b : 4 * b + 4, :]
                    d1 = p1 - (P0A if kind != "l" else P0L)
                    nc.tensor.matmul(out_sl, lhsT=wsl(0, h),
                                     rhs=_ktjump(tl, o0, da, 512),
                                     start=True, stop=False, perf_mode=DR)
                    nc.tensor.matmul(out_sl, lhsT=wsl(1, h),
                                     rhs=_ktjump(tl, o0 + d1, db, 512),
                                     start=False, stop=True, perf_mode=DR)
                g_t = gpool.tile([128, 8, 128], BF16, tag="g", name=f"g{kind}{gi}_{h}")
                nc.scalar.activation(out=g_t, in_=ps, func=GELU,
                                     bias=b1_ap(h), scale=1.0)
                pending.append((g_t, kind, gi, h))
                if len(pending) > 2:
                    emit_scores(*pending.pop(0))
        while pending:
            emit_scores(*pending.pop(0))

        # ---------- epilogue ----------
        if len(set(b2_scaled)) == 1:
            nc.scalar.activation(out=pt_sb, in_=score_ps, func=EXP,
                                 bias=float(b2_scaled[0]), scale=SCALE)
        else:
            for h in range(H):
                nc.scalar.activation(out=pt_sb[:, :, h, :],
                                     in_=score_ps[:, :, h, :],
                                     func=EXP, bias=float(b2_scaled[h]),
                                     scale=SCALE)
        nc.vector.tensor_mul(pt_sb, pt_sb, mask_bc)

        av = psepi.tile([IBLK, H, HS + 1], F32, tag="av", name="av")
        for h in range(H):
            for jc in range(2):
                nc.tensor.matmul(av[:, h, :], lhsT=pt_sb[:, jc, h, :],
                                 rhs=v_sb[:, jc, h, :],
                                 start=(jc == 0), stop=(jc == 1))
        nc.vector.reciprocal(out=recip, in_=av[:, :, HS : HS + 1])
        rb = recip[:, :, 0:1]
        recip_bc = bass.AP(tensor=rb.tensor, offset=rb.offset,
                           ap=[list(rb.ap[0]), [1, H], [0, HS]])
        fview = bass.AP(tensor=final_sb.tensor, offset=final_sb.offset,
                        ap=[list(final_sb.ap[0]), [HS, H], [1, HS]])
        nc.vector.tensor_mul(fview, av[:, :, 0:HS], recip_bc)
        nc.sync.dma_start(out=out[:], in_=final_sb)

    return nc


def _prep_core(x1t_b, pd_b, k):
    """Build the per-core big-tile columns (fp8) and mask for residue k."""
    f8 = ml_dtypes.float8_e4m3fn
    qsel = 4 * np.arange(IBLK) + k
    arr = pd_b[qsel].transpose(2, 0, 1)            # (256 c2, 64 q, 256 j)
    kt0, kt1 = arr[0:128], arr[128:256]
    xs0 = kt0[:, 0:16, 0:64].reshape(128, 1024)
    xs1 = kt1[:, 0:16, 0:64].reshape(128, 1024)
    s0 = kt0[:, 16:32, 0:128].reshape(128, 2048)
    s1 = kt1[:, 16:32, 0:128].reshape(128, 2048)
    l0 = kt0[:, 32:64, :].reshape(128, 8192)
    l1 = kt1[:, 32:64, :].reshape(128, 8192)
    x1qs = np.ascontiguousarray(x1t_b[:, qsel]).astype(f8)   # (128, 64)
    xt64 = np.tile(x1t_b[:, 0:64], (1, 8))
    xt128 = np.tile(x1t_b[:, 0:128], (1, 4))
    xt256 = np.tile(x1t_b, (1, 2))
    bigc = np.concatenate(
        [xs0, s0, xt64, xt128, xs1, s1, l0, xt256, l1], axis=1).astype(f8)
    jidx = np.arange(128)[:, None, None] + np.array([0, 128])[None, :, None]
    mask = (jidx <= (4 * np.arange(IBLK) + k)[None, None, :]).astype(
        ml_dtypes.bfloat16)
    return bigc, x1qs, mask


def kernel(**inputs):
    x = np.asarray(inputs["x"], np.float32)
    st = np.asarray(inputs["st_pos_emb"], np.float32)
    pd = np.asarray(inputs["pos_dist_emb"], np.float32)
    W1 = np.asarray(inputs["W1"], np.float32)
    b1 = np.asarray(inputs["b1"], np.float32)
    W2 = np.asarray(inputs["W2"], np.float32)
    b2 = np.asarray(inputs["b2"], np.float32)
    Wv = np.asarray(inputs["Wv"], np.float32)
    bv = np.asarray(inputs["bv"], np.float32)

    bf = ml_dtypes.bfloat16
    f8 = ml_dtypes.float8_e4m3fn
    x1 = x + st[None]                                    # (B, T, C)
    x1t_b = np.ascontiguousarray(x1.transpose(0, 2, 1))  # (B, C, T)

    W1k = W1[:, :C, :]                                   # (H, C, C)
    W1q = W1[:, C : 2 * C, :]
    W1p = W1[:, 2 * C :, :]                              # (H, 2C, C)
    wa_a = np.stack([W1p[:, 0:128, :], W1k], axis=0)     # (kt, H, c, m)
    wb_a = np.stack([W1p[:, 128:256, :], W1q], axis=0)
    wab_a = np.ascontiguousarray(
        np.stack([wa_a, wb_a], axis=0).transpose(3, 0, 1, 2, 4)
    ).astype(f8)                                         # (c, a/b, kt, h, m)
    w2_a = np.ascontiguousarray(W2.T).astype(bf)         # (C, H)
    b1_a = np.ascontiguousarray(b1.T)                    # (C, H)
    wv_a = Wv.transpose(1, 0, 2).reshape(C, H * HS).astype(bf)

    key = tuple(float(v) * SCALE for v in b2)
    if key not in _build_cache:
        _build_cache[key] = _build(key)
    nc = _build_cache[key]

    in_maps = []
    for core in range(NCORES):
        b, k = divmod(core, 4)
        bigc, x1qs, mask = _prep_core(x1t_b[b], pd[b], k)
        cpack = np.concatenate(
            [w2_a, wv_a, x[b].T.astype(bf), mask.reshape(128, 128)], axis=1)
        wabq = np.concatenate(
            [wab_a.reshape(128, 2048), x1qs, w2_a.view(f8), b1_a.view(f8)],
            axis=1)
        in_maps.append({
            "big": bigc, "wabq": np.ascontiguousarray(wabq),
            "cpack": np.ascontiguousarray(cpack),
        })

    _install_wait_legalizer()
    res = run_bass_kernel_spmd(nc, in_maps, core_ids=list(range(NCORES)))
    outp = np.zeros((B, T, H * HS), np.float32)
    for core in range(NCORES):
        b, k = divmod(core, 4)
        outp[b, 4 * np.arange(IBLK) + k] = res.results[core]["out"]
    outp += bv.reshape(-1)[None, None, :]
    return outp
